# revision 20
# baseline (speedup 1.0000x reference)
"""Trainium2 Bass kernel: multi-head attention with RoPE + gated prompt
injection (nn_Attention_28080496181816), sharded over 8 NeuronCores.

Sharding: tensor-parallel over heads. Core c owns heads [4c, 4c+4):
  - wq/wk/wv column-sharded (per-head), o-proj via AllGather of the
    per-core attention outputs + column-sharded wo matmul.
  - Host-side unshard is a pure concatenation of output column slices.

Layout: "T-major" — activations live as [feature, token] on device so
every matmul contraction lands on the partition axis with no on-device
transposes. RoPE pairs are made contiguous by permuting wq/wk rows
(per head: even hd dims then odd hd dims) on the host.

Attention runs in 512-wide query groups with variable-width score
matmuls derived from the runtime mask structure (causal -> exact
lower-triangle work). The emission order software-pipelines phases so
the in-order PE stream interleaves attention with the next batch's
projections and the previous batch's output projection.
"""

import math
import os
import sys
import types

import numpy as np
import ml_dtypes

# --- optional NTFF profile hook shim (only needed if BASS_TRACE is set;
# the stock image lacks antenv.axon_hooks) ---
try:
    import antenv.axon_hooks  # noqa: F401
except Exception:
    try:
        import antenv
        _m = types.ModuleType("antenv.axon_hooks")
        _hook = [None]
        _m.set_axon_ntff_profile_hook = lambda h: _hook.__setitem__(0, h)
        _m.get_axon_ntff_profile_hook = lambda: _hook[0]
        sys.modules["antenv.axon_hooks"] = _m
        antenv.axon_hooks = _m
        from trn_agent_boot.trn_boot import _ntff_profile_via_ctypes
        _p = _ntff_profile_via_ctypes("/opt/axon/libaxon_pjrt.so")
        if _p is not None:
            _m.set_axon_ntff_profile_hook(_p)
    except Exception:
        pass

import concourse.bacc as bacc
import concourse.mybir as mybir
import concourse.tile as tile
from concourse import bass_utils

# If tracing is enabled in an environment without an artifact bucket,
# don't let the upload step crash the run.
_orig_upload = bass_utils.upload_artifacts


def _safe_upload(tmpdir):
    try:
        return _orig_upload(tmpdir)
    except Exception:
        return tmpdir


bass_utils.upload_artifacts = _safe_upload

BF16 = mybir.dt.bfloat16
F32 = mybir.dt.float32
NPBF16 = ml_dtypes.bfloat16

B, S, D, H, HD, PL = 2, 1024, 4096, 32, 128, 10
NC = 8              # cores
HLOC = H // NC      # 4 heads per core
DLOC = HLOC * HD    # 512
T = B * S           # 2048
NDX = D // 128      # 32 contraction blocks
NQT = S // 128      # 8 query tiles per batch
NQG = NQT // 4      # 2 query groups of 512
SCALE = 1.0 / math.sqrt(HD)

_PROG_CACHE = {}


def _analyze_mask(mask):
    """Classify each 128x128 tile of the additive mask: skip (fully
    masked), clear (all zero) or mixed (ship the transposed, pre-scaled
    tile). Deduplicates mixed tiles."""
    mq = np.asarray(mask).reshape(S, S)
    plan = []
    uniq = {}
    mlist = []
    for qi in range(NQT):
        row = []
        for kb in range(NQT):
            sub = mq[qi * 128:(qi + 1) * 128, kb * 128:(kb + 1) * 128]
            if np.all(sub <= -1e8):
                continue
            if np.all(sub == 0):
                row.append((kb, None))
                continue
            tt = np.ascontiguousarray(sub.T.astype(np.float32) / SCALE)
            key = tt.tobytes()
            if key not in uniq:
                uniq[key] = len(mlist)
                mlist.append(tt)
            row.append((kb, uniq[key]))
        plan.append(row)
    return plan, mlist


def _group_plan(plan, n_mtiles):
    """512-wide query groups. Per group: list of (kb, q0, q1, adds) with
    q0..q1 the covered query quarters and adds = [(quarter, mtile_idx)];
    mtile_idx == n_mtiles selects the -inf tile. The first kb of each
    group always spans the full group so PSUM has_written is set."""
    NEG = n_mtiles
    plan2 = []
    for qg in range(NQG):
        qmode = []
        for q in range(4):
            qmode.append(dict(plan[qg * 4 + q]))
        live = sorted(set().union(*[set(d.keys()) for d in qmode]))
        entries = []
        for j, kb in enumerate(live):
            pres = [kb in qmode[q] for q in range(4)]
            if j == 0:
                q0, q1 = 0, 3
            else:
                q0 = min(q for q in range(4) if pres[q])
                q1 = max(q for q in range(4) if pres[q])
            adds = []
            for q in range(q0, q1 + 1):
                if not pres[q]:
                    adds.append((q, NEG))
                elif qmode[q][kb] is not None:
                    adds.append((q, qmode[q][kb]))
            entries.append((kb, q0, q1, tuple(adds)))
        plan2.append(tuple(entries))
    return plan2


def _build_program(plan2, n_mt):
    """Build + compile the SPMD program (identical on all 8 cores).
    n_mt counts mask tiles INCLUDING the trailing -inf tile."""
    nc = bacc.Bacc("TRN2", target_bir_lowering=False, debug=False, num_devices=NC)

    # p-major host layouts so each logical group is ONE big DMA
    xt = nc.dram_tensor("xt", [4, 128, NDX, 512], BF16, kind="ExternalInput")
    wqt = nc.dram_tensor("wqt", [HLOC, 128, NDX, 128], BF16, kind="ExternalInput")
    wkt = nc.dram_tensor("wkt", [HLOC, 128, NDX, 128], BF16, kind="ExternalInput")
    wvt = nc.dram_tensor("wvt", [128, NDX, DLOC], BF16, kind="ExternalInput")
    wot = nc.dram_tensor("wot", [128, NDX, DLOC], BF16, kind="ExternalInput")
    pt = nc.dram_tensor("pt", [128, NDX, PL], BF16, kind="ExternalInput")
    cosT = nc.dram_tensor("cosT", [64, S], F32, kind="ExternalInput")
    sinT = nc.dram_tensor("sinT", [64, S], F32, kind="ExternalInput")
    gates = nc.dram_tensor("gates", [PL, HLOC], F32, kind="ExternalInput")
    mtiles = nc.dram_tensor("mtiles", [n_mt, 128, 128], BF16, kind="ExternalInput")
    ident = nc.dram_tensor("ident", [128, 128], BF16, kind="ExternalInput")
    out_d = nc.dram_tensor("out", [T, DLOC], F32, kind="ExternalOutput")

    AF = mybir.ActivationFunctionType
    OP = mybir.AluOpType
    labels = {}
    nc._unit_labels = labels

    def _lb(inst, tag):
        labels[inst.ins.name] = tag
        return inst

    with tile.TileContext(nc) as tc:
        with (
            tc.tile_pool(name="const", bufs=1) as cpool,
            tc.tile_pool(name="wres", bufs=1) as wres,
            tc.tile_pool(name="stream", bufs=1) as sp,
            tc.tile_pool(name="act", bufs=1) as ap,
            tc.tile_pool(name="psum", bufs=1, space="PSUM") as pp,
            tc.tile_pool(name="dram", bufs=1, space="DRAM") as dp,
        ):
            # ---- persistent constants / weights ----
            cos_sb = cpool.tile([64, S], F32, tag="cos")
            sin_sb = cpool.tile([64, S], F32, tag="sin")
            nc.gpsimd.dma_start(cos_sb[:], cosT[:])
            nc.gpsimd.dma_start(sin_sb[:], sinT[:])
            gates_sb = cpool.tile([PL, HLOC], F32, tag="gates")
            nc.gpsimd.dma_start(gates_sb[:], gates[:])
            mt_sb = []
            for i in range(n_mt):
                t = cpool.tile([128, 128], BF16, tag=f"mt{i}", name=f"mt{i}")
                nc.gpsimd.dma_start(t[:], mtiles[i])
                mt_sb.append(t)
            id_sb = cpool.tile([128, 128], BF16, tag="ident")
            nc.gpsimd.dma_start(id_sb[:], ident[:])
            ones_col = cpool.tile([128, 1], BF16, tag="ones_col")
            nc.vector.memset(ones_col[:], 1.0)

            # wv / wo resident: 4 tiles each of [128, 8*512]
            # (DMAs issued on the GpSimd queue after the first projection
            # group so they don't delay the critical-path x/wq loads)
            wv_sb = [wres.tile([128, 8 * DLOC], BF16, tag=f"wv{j}",
                               name=f"wv{j}") for j in range(4)]
            wo_sb = [wres.tile([128, 8 * DLOC], BF16, tag=f"wo{j}",
                               name=f"wo{j}") for j in range(4)]

            def emit_wvwo_loads():
                for j in range(4):
                    nc.gpsimd.dma_start(wv_sb[j][:], wvt[:, 8 * j:8 * (j + 1), :])
                for j in range(4):
                    nc.gpsimd.dma_start(wo_sb[j][:], wot[:, 8 * j:8 * (j + 1), :])

            def wv_sl(i):
                return wv_sb[i // 8][:, (i % 8) * DLOC:(i % 8 + 1) * DLOC]

            def wo_sl(i):
                return wo_sb[i // 8][:, (i % 8) * DLOC:(i % 8 + 1) * DLOC]

            pt_sb = cpool.tile([128, NDX * PL], BF16, tag="pt")
            nc.gpsimd.dma_start(pt_sb[:], pt[:])

            pk_sb = [ap.tile([128, PL], BF16, tag=f"pk{h}", name=f"pk{h}")
                     for h in range(HLOC)]
            pv_sb = ap.tile([PL, DLOC], BF16, tag="pv")

            # b=0: one AG per head; b=1 qg0: one AG per head-pair; b=1 qg1:
            # one AG per head so the tail collective after the last
            # attention unit is only 128KB and lands quickly.
            agin = {}
            agout = {}
            for h in range(HLOC):
                agin[0, h] = dp.tile([NQT, 128, 128], BF16,
                                     tag=f"agin0_{h}", name=f"agin0_{h}")
                agout[0, h] = dp.tile([NC, NQT, 128, 128], BF16,
                                      tag=f"agout0_{h}", name=f"agout0_{h}",
                                      addr_space="Shared")
            for hf in range(2):
                agin[1, 0, hf] = dp.tile([2, 4, 128, 128], BF16,
                                         tag=f"agin1_0_{hf}",
                                         name=f"agin1_0_{hf}")
                agout[1, 0, hf] = dp.tile([NC, 2, 4, 128, 128], BF16,
                                          tag=f"agout1_0_{hf}",
                                          name=f"agout1_0_{hf}",
                                          addr_space="Shared")
            aginh = {}
            agouth = {}
            for h in range(HLOC):
                aginh[h] = dp.tile([4, 128, 128], BF16, tag=f"aginh{h}",
                                   name=f"aginh{h}")
                agouth[h] = dp.tile([NC, 4, 128, 128], BF16, tag=f"agouth{h}",
                                    name=f"agouth{h}", addr_space="Shared")

            XT_BUFS = 4     # [128, 4096] quarters (one chunk live)
            WQK_BUFS = 2
            QK_BUFS = 6
            V_BUFS = NQT + 4
            AG_BUFS = 2

            qT = {}
            kT = {}
            v_sb = {}

            def gen_qkv(b, chunks=(0, 1)):
                for tc2 in chunks:
                    tcg = b * 2 + tc2
                    cols = slice(tc2 * 512, (tc2 + 1) * 512)
                    xts = [sp.tile([128, 8 * 512], BF16, tag="xt",
                                   bufs=XT_BUFS, name=f"xt{tcg}_{q}")
                           for q in range(4)]
                    if tcg == 0:
                        for pp_ in range(4):
                            nc.sync.dma_start(
                                xts[0][:, pp_ * 1024:(pp_ + 1) * 1024],
                                xt[tcg, :, 2 * pp_:2 * (pp_ + 1), :])
                    else:
                        nc.sync.dma_start(xts[0][:, 0:2048], xt[tcg, :, 0:4, :])
                        nc.sync.dma_start(xts[0][:, 2048:4096], xt[tcg, :, 4:8, :])
                    xlate = [(q, xts[q]) for q in range(1, 4)]

                    def x_sl(i):
                        return xts[i // 8][:, (i % 8) * 512:(i % 8 + 1) * 512]

                    if tc2 == 0:
                        qT[b] = [sp.tile([128, S], BF16, tag="qT", bufs=QK_BUFS,
                                         name=f"qT{b}_{j}") for j in range(HLOC)]
                        kT[b] = [sp.tile([128, S], BF16, tag="kT", bufs=QK_BUFS,
                                         name=f"kT{b}_{j}") for j in range(HLOC)]
                        v_sb[b] = [sp.tile([128, DLOC], BF16, tag="v", bufs=V_BUFS,
                                           name=f"v{b}_{j}") for j in range(NQT)]
                    # --- q & k projections (T-major out) + RoPE ---
                    for proj, wdram, dstT in ((0, wqt, qT[b]), (1, wkt, kT[b])):
                        for dqb in range(HLOC):
                            wt = sp.tile([128, NDX * 128], BF16, tag="wqk",
                                         bufs=WQK_BUFS)
                            wt_eng = nc.scalar if b == 0 else nc.sync
                            wt_eng.dma_start(wt[:, 0:2048],
                                             wdram[dqb, :, 0:16, :])
                            wt_eng.dma_start(wt[:, 2048:4096],
                                             wdram[dqb, :, 16:32, :])
                            while xlate:
                                q, xtile = xlate.pop(0)
                                wt_eng.dma_start(
                                    xtile[:], xt[tcg, :, 8 * q:8 * (q + 1), :])
                            ps = pp.tile([128, 512], F32, tag="mm512", bufs=2)
                            for i in range(NDX):
                                _lb(nc.tensor.matmul(
                                    ps[:], wt[:, i * 128:(i + 1) * 128], x_sl(i),
                                    start=(i == 0), stop=(i == NDX - 1)),
                                    f"qkv{b}.{tc2}.p{proj}.d{dqb}.{i}")
                            if proj == 1 and b == 0 and tc2 == 0:
                                # prompt keys for this head, reusing wk tiles
                                psk = pp.tile([128, 512], F32, tag="sc", bufs=3)
                                for i in range(NDX):
                                    nc.tensor.matmul(
                                        psk[:, 0:PL], wt[:, i * 128:(i + 1) * 128],
                                        pt_sb[:, i * PL:(i + 1) * PL],
                                        start=(i == 0), stop=(i == NDX - 1))
                                nc.vector.tensor_copy(pk_sb[dqb][:], psk[:, 0:PL])
                            # RoPE: rows 0:64 = even hd dims, 64:128 = odd
                            c_sl = cos_sb[:, cols]
                            s_sl = sin_sb[:, cols]
                            t_rc = sp.tile([64, 512], BF16, tag="rt", bufs=4)
                            t_rs = sp.tile([64, 512], BF16, tag="rt", bufs=4)
                            t_ic = sp.tile([64, 512], BF16, tag="rt", bufs=4)
                            t_is = sp.tile([64, 512], BF16, tag="rt", bufs=4)
                            nc.vector.tensor_tensor(t_rc[:], ps[0:64, :], c_sl, op=OP.mult)
                            nc.vector.tensor_tensor(t_rs[:], ps[0:64, :], s_sl, op=OP.mult)
                            nc.vector.tensor_tensor(t_ic[:], ps[64:128, :], c_sl, op=OP.mult)
                            nc.vector.tensor_tensor(t_is[:], ps[64:128, :], s_sl, op=OP.mult)
                            nc.vector.tensor_tensor(dstT[dqb][0:64, cols], t_rc[:],
                                                    t_is[:], op=OP.subtract)
                            nc.gpsimd.tensor_tensor(dstT[dqb][64:128, cols], t_rs[:],
                                                    t_ic[:], op=OP.add)
                            yield
                    # --- v projection (natural [t, dv]) ---
                    for tblk in range(4):
                        ps = pp.tile([128, 512], F32, tag="mm512", bufs=2)
                        for i in range(NDX):
                            _lb(nc.tensor.matmul(
                                ps[:], x_sl(i)[:, tblk * 128:(tblk + 1) * 128],
                                wv_sl(i), start=(i == 0), stop=(i == NDX - 1)),
                                f"v{b}.{tc2}.{tblk}.{i}")
                        nc.vector.tensor_copy(v_sb[b][tc2 * 4 + tblk][:], ps[:])
                        yield
                    if b == 0 and tc2 == 0:
                        psv = pp.tile([128, 512], F32, tag="mm512", bufs=2)
                        for i in range(NDX):
                            nc.tensor.matmul(psv[0:PL, :],
                                             pt_sb[:, i * PL:(i + 1) * PL],
                                             wv_sl(i),
                                             start=(i == 0), stop=(i == NDX - 1))
                        nc.vector.tensor_copy(pv_sb[:], psv[0:PL, :])
                        for hh in range(HLOC):
                            nc.vector.tensor_scalar(
                                pv_sb[0:PL, hh * 128:(hh + 1) * 128],
                                pv_sb[0:PL, hh * 128:(hh + 1) * 128],
                                gates_sb[0:PL, hh:hh + 1], None, op0=OP.mult)

            def gen_att(b):
                if b == 0:
                    hq_order = [(h, qg) for h in range(HLOC) for qg in range(NQG)]
                else:
                    # qg-outer so every head's qg0 AllGather lands mid-phase
                    hq_order = [(h, qg) for qg in range(NQG) for h in range(HLOC)]
                for h, qg in hq_order:
                    if True:
                        stage = sp.tile([128, 512], BF16, tag="stage", bufs=2,
                                        name=f"stage{b}_{h}_{qg}")
                        qbase = qg * 512
                        entries = plan2[qg]
                        probs = []
                        for kb, q0, q1, adds in entries:
                            coff = q0 * 128
                            ncols = (q1 - q0 + 1) * 128
                            ssc = pp.tile([128, 512], F32, tag="sc", bufs=3)
                            _lb(nc.tensor.matmul(
                                ssc[:, coff:coff + ncols],
                                kT[b][h][:, kb * 128:(kb + 1) * 128],
                                qT[b][h][:, qbase + coff:qbase + coff + ncols],
                                start=True, stop=(not adds)),
                                f"sc{b}.h{h}.g{qg}.k{kb}")
                            for ai, (q, idx) in enumerate(adds):
                                nc.tensor.matmul(
                                    ssc[:, q * 128:(q + 1) * 128], id_sb[:],
                                    mt_sb[idx][:], start=False,
                                    stop=(ai == len(adds) - 1))
                            pr = sp.tile([128, 512], BF16, tag="probs", bufs=9)
                            nc.scalar.activation(pr[:, coff:coff + ncols],
                                                 ssc[:, coff:coff + ncols],
                                                 AF.Exp, scale=SCALE)
                            probs.append((kb, coff, ncols, pr))
                        # prompt scores
                        psc = pp.tile([128, 512], F32, tag="sc", bufs=3)
                        nc.tensor.matmul(psc[0:PL, :], pk_sb[h][:],
                                         qT[b][h][:, qbase:qbase + 512],
                                         start=True, stop=True)
                        ppr = sp.tile([PL, 512], BF16, tag="pprobs", bufs=1)
                        nc.scalar.activation(ppr[:], psc[0:PL, :], AF.Exp,
                                             scale=SCALE)
                        # PV accumulation + sums
                        po = pp.tile([128, 512], F32, tag="pv", bufs=2)
                        pss = pp.tile([128, 512], F32, tag="aux", bufs=1)
                        n = len(probs)
                        for i, (kb, coff, ncols, pr) in enumerate(probs):
                            _lb(nc.tensor.matmul(
                                po[:, coff:coff + ncols],
                                v_sb[b][kb][:, h * 128:(h + 1) * 128],
                                pr[:, coff:coff + ncols],
                                start=(i == 0), stop=(i == n - 1)),
                                f"pv{b}.h{h}.g{qg}.k{kb}")
                        for i, (kb, coff, ncols, pr) in enumerate(probs):
                            nc.tensor.matmul(
                                pss[0:1, coff:coff + ncols], ones_col[:, 0:1],
                                pr[:, coff:coff + ncols],
                                start=(i == 0), stop=(i == n - 1))
                        ppo = pp.tile([128, 512], F32, tag="pv", bufs=2)
                        nc.tensor.matmul(ppo[:], pv_sb[0:PL, h * 128:(h + 1) * 128],
                                         ppr[:], start=True, stop=True)
                        nc.tensor.matmul(pss[32:33, :], ones_col[0:PL, 0:1], ppr[:],
                                         start=True, stop=True)
                        # 1/s on the Vector engine (frees ScalarE for exp)
                        recs = sp.tile([1, 1024], F32, tag="recs", bufs=1)
                        nc.vector.reciprocal(recs[0:1, 0:512], pss[0:1, :])
                        nc.vector.reciprocal(recs[0:1, 512:1024], pss[32:33, :])
                        recs_b = sp.tile([1, 1024], BF16, tag="recsb", bufs=1)
                        nc.vector.tensor_copy(recs_b[:], recs[:])
                        # broadcast row-vector across partitions (GpSimd)
                        bcs = sp.tile([128, 1024], BF16, tag="bcs", bufs=2)
                        nc.gpsimd.partition_broadcast(bcs[:], recs_b[0:1, :])
                        # PSUM->SBUF on ScalarE (closest to PSUM; DVE PSUM
                        # reads measure far slower than their SBUF tier)
                        po_c = sp.tile([128, 512], BF16, tag="poc", bufs=2)
                        ppo_c = sp.tile([128, 512], BF16, tag="poc", bufs=2)
                        nc.scalar.copy(po_c[:], po[:])
                        nc.scalar.copy(ppo_c[:], ppo[:])
                        t1 = sp.tile([128, 512], BF16, tag="cmb", bufs=2)
                        t2 = sp.tile([128, 512], BF16, tag="cmb", bufs=2)
                        nc.gpsimd.tensor_tensor(t1[:], po_c[:], bcs[:, 0:512],
                                                op=OP.mult)
                        nc.gpsimd.tensor_tensor(t2[:], ppo_c[:], bcs[:, 512:1024],
                                                op=OP.mult)
                        nc.gpsimd.tensor_tensor(stage[:], t1[:], t2[:], op=OP.add)
                        if b == 0:
                            nc.sync.dma_start(
                                agin[0, h][qg * 4:(qg + 1) * 4].rearrange(
                                    "n p c -> p n c"), stage[:])
                            if qg == NQG - 1:
                                nc.gpsimd.collective_compute(
                                    "AllGather", OP.bypass,
                                    replica_groups=[list(range(NC))],
                                    ins=[agin[0, h].opt()],
                                    outs=[agout[0, h].opt()])
                        elif qg == 0:
                            nc.sync.dma_start(
                                agin[1, 0, h // 2][h % 2].rearrange(
                                    "n p c -> p n c"), stage[:])
                            if h % 2 == 1:
                                nc.gpsimd.collective_compute(
                                    "AllGather", OP.bypass,
                                    replica_groups=[list(range(NC))],
                                    ins=[agin[1, 0, h // 2].opt()],
                                    outs=[agout[1, 0, h // 2].opt()])
                        else:
                            nc.sync.dma_start(
                                aginh[h].rearrange("n p c -> p n c"), stage[:])
                            nc.gpsimd.collective_compute(
                                "AllGather", OP.bypass,
                                replica_groups=[list(range(NC))],
                                ins=[aginh[h].opt()],
                                outs=[agouth[h].opt()])
                        yield

            def _agt_fetch(b, tq):
                agt = sp.tile([128, NDX * 128], BF16, tag="ag",
                              bufs=AG_BUFS, name=f"ag{b}_{tq}")
                agt_v = agt[:].rearrange("p (n h c) -> p n h c",
                                         n=NC, h=HLOC, c=128)
                for hl in range(HLOC):
                    if b == 0:
                        src = agout[0, hl][:, tq]
                    elif tq < 4:
                        src = agout[1, 0, hl // 2][:, hl % 2, tq % 4]
                    else:
                        src = agouth[hl][:, tq % 4]
                    eng = nc.sync if hl % 2 == 0 else nc.gpsimd
                    eng.dma_start(agt_v[:, :, hl, :],
                                  src.rearrange("n p c -> p n c"))
                return agt

            def gen_oproj(b, tqs):
                for tq in tqs:
                    agt = _agt_fetch(b, tq)
                    pso = pp.tile([128, 512], F32, tag="mm512", bufs=2)
                    order = [core * HLOC + hl for hl in range(HLOC)
                             for core in range(NC)]
                    for j, i in enumerate(order):
                        _lb(nc.tensor.matmul(
                            pso[:], agt[:, i * 128:(i + 1) * 128],
                            wo_sl(i), start=(j == 0), stop=(j == NDX - 1)),
                            f"op{b}.t{tq}.{i}")
                    ost = sp.tile([128, 512], F32, tag="ost", bufs=1)
                    nc.vector.tensor_copy(ost[:], pso[:])
                    r0 = b * S + tq * 128
                    nc.sync.dma_start(out_d[r0:r0 + 128, :], ost[:])
                    yield

            # ---- software-pipelined emission ----
            g_qkv0 = gen_qkv(0)
            next(g_qkv0)
            emit_wvwo_loads()
            for _ in g_qkv0:
                pass
            g_att0, g_qkv1 = gen_att(0), gen_qkv(1)
            for _ in g_att0:
                for _ in range(3):
                    next(g_qkv1, None)
            for _ in g_qkv1:
                pass
            # att1: interleave most of o0; hold back the rest of o0 plus all
            # of o1 for the tail, which is then pure PE work that covers the
            # last collectives' wire time.
            g_att1 = gen_att(1)
            g_o0 = gen_oproj(0, list(range(NQT)))
            g_o1 = gen_oproj(1, list(range(NQT)))
            cnt = 0
            for _ in g_att1:
                cnt += 1
                if cnt >= 4:
                    next(g_o0, None)
            for _ in g_o0:
                pass
            for _ in g_o1:
                pass

    nc.compile()
    return nc


def kernel(**inputs):
    x = np.asarray(inputs["x"], np.float32)
    wq = np.asarray(inputs["wq"], np.float32)
    wk = np.asarray(inputs["wk"], np.float32)
    wv = np.asarray(inputs["wv"], np.float32)
    wo = np.asarray(inputs["wo"], np.float32)
    prompt = np.asarray(inputs["prompt"], np.float32)
    prompt_gate = np.asarray(inputs["prompt_gate"], np.float32)
    freqs_cos = np.asarray(inputs["freqs_cos"], np.float32)
    freqs_sin = np.asarray(inputs["freqs_sin"], np.float32)
    mask = np.asarray(inputs["mask"], np.float32)

    plan, mlist = _analyze_mask(mask)
    plan2 = _group_plan(plan, len(mlist))
    n_mt = len(mlist) + 1  # + trailing -inf tile
    plan_key = (tuple(plan2), n_mt)
    if plan_key not in _PROG_CACHE:
        _PROG_CACHE[plan_key] = _build_program(plan2, n_mt)
    nc = _PROG_CACHE[plan_key]

    # ---- shared host prep ----
    perm = np.concatenate([np.arange(0, HD, 2), np.arange(1, HD, 2)])
    xT = np.ascontiguousarray(x.reshape(T, D).T.astype(NPBF16))
    # [4, 128, NDX, 512]: [tcg, dx_in_block, dx_block, t_in_chunk]
    xt_tiles = np.ascontiguousarray(
        xT.reshape(NDX, 128, 4, 512).transpose(2, 1, 0, 3))
    ptT = np.ascontiguousarray(prompt.T.astype(NPBF16))       # [D, PL]
    pt_tiles = np.ascontiguousarray(
        ptT.reshape(NDX, 128, PL).transpose(1, 0, 2))
    cosT = np.ascontiguousarray(freqs_cos.T.astype(np.float32))
    sinT = np.ascontiguousarray(freqs_sin.T.astype(np.float32))
    neg = np.full((1, 128, 128), -1e30, np.float32)
    if mlist:
        mtiles = np.concatenate([np.stack(mlist), neg]).astype(NPBF16)
    else:
        mtiles = neg.astype(NPBF16)

    def shard_qk(w, c):
        rows = np.concatenate(
            [c * DLOC + j * HD + perm for j in range(HLOC)])
        wT = w[rows, :].T.astype(NPBF16)                      # [D, DLOC]
        return np.ascontiguousarray(
            wT.reshape(NDX, 128, HLOC, 128).transpose(2, 1, 0, 3))

    def shard_rhs(w, c):
        # rows c*DLOC..+DLOC of w, transposed -> [D, DLOC] -> [128,NDX,DLOC]
        wT = w[c * DLOC:(c + 1) * DLOC, :].T.astype(NPBF16)
        return np.ascontiguousarray(wT.reshape(NDX, 128, DLOC).transpose(1, 0, 2))

    in_maps = []
    for c in range(NC):
        in_maps.append(dict(
            xt=xt_tiles,
            wqt=shard_qk(wq, c),
            wkt=shard_qk(wk, c),
            wvt=shard_rhs(wv, c),
            wot=shard_rhs(wo, c),
            pt=pt_tiles,
            cosT=cosT,
            sinT=sinT,
            gates=np.ascontiguousarray(np.repeat(
                prompt_gate.reshape(H)[c * HLOC:(c + 1) * HLOC][None, :],
                PL, axis=0)).astype(np.float32),
            mtiles=mtiles,
            ident=np.eye(128, dtype=NPBF16),
        ))

    res = bass_utils.run_bass_kernel_spmd(
        nc, in_maps, core_ids=list(range(NC)),
        trace=bool(os.environ.get("BASS_TRACE")))
    kernel.last_result = res

    full = np.empty((T, D), np.float32)
    for c in range(NC):
        full[:, c * DLOC:(c + 1) * DLOC] = res.results[c]["out"]
    return full.reshape(B, S, D)


# revision 21
# speedup vs baseline: 1.0506x; 1.0506x over previous
"""Trainium2 Bass kernel: multi-head attention with RoPE + gated prompt
injection (nn_Attention_28080496181816), sharded over 8 NeuronCores.

Sharding: tensor-parallel over heads. Core c owns heads [4c, 4c+4):
  - wq/wk/wv column-sharded (per-head), o-proj via AllGather of the
    per-core attention outputs + column-sharded wo matmul.
  - Host-side unshard is a pure concatenation of output column slices.

Layout: "T-major" — activations live as [feature, token] on device so
every matmul contraction lands on the partition axis with no on-device
transposes. RoPE pairs are made contiguous by permuting wq/wk rows
(per head: even hd dims then odd hd dims) on the host.

Attention runs in 512-wide query groups with variable-width score
matmuls derived from the runtime mask structure (causal -> exact
lower-triangle work). The emission order software-pipelines phases so
the in-order PE stream interleaves attention with the next batch's
projections and the previous batch's output projection.
"""

import math
import os
import sys
import types

import numpy as np
import ml_dtypes

# --- optional NTFF profile hook shim (only needed if BASS_TRACE is set;
# the stock image lacks antenv.axon_hooks) ---
try:
    import antenv.axon_hooks  # noqa: F401
except Exception:
    try:
        import antenv
        _m = types.ModuleType("antenv.axon_hooks")
        _hook = [None]
        _m.set_axon_ntff_profile_hook = lambda h: _hook.__setitem__(0, h)
        _m.get_axon_ntff_profile_hook = lambda: _hook[0]
        sys.modules["antenv.axon_hooks"] = _m
        antenv.axon_hooks = _m
        from trn_agent_boot.trn_boot import _ntff_profile_via_ctypes
        _p = _ntff_profile_via_ctypes("/opt/axon/libaxon_pjrt.so")
        if _p is not None:
            _m.set_axon_ntff_profile_hook(_p)
    except Exception:
        pass

import concourse.bacc as bacc
import concourse.mybir as mybir
import concourse.tile as tile
from concourse import bass_utils

# If tracing is enabled in an environment without an artifact bucket,
# don't let the upload step crash the run.
_orig_upload = bass_utils.upload_artifacts


def _safe_upload(tmpdir):
    try:
        return _orig_upload(tmpdir)
    except Exception:
        return tmpdir


bass_utils.upload_artifacts = _safe_upload

BF16 = mybir.dt.bfloat16
F32 = mybir.dt.float32
NPBF16 = ml_dtypes.bfloat16

B, S, D, H, HD, PL = 2, 1024, 4096, 32, 128, 10
NC = 8              # cores
HLOC = H // NC      # 4 heads per core
DLOC = HLOC * HD    # 512
T = B * S           # 2048
NDX = D // 128      # 32 contraction blocks
NQT = S // 128      # 8 query tiles per batch
NQG = NQT // 4      # 2 query groups of 512
SCALE = 1.0 / math.sqrt(HD)

_PROG_CACHE = {}


def _analyze_mask(mask):
    """Classify each 128x128 tile of the additive mask: skip (fully
    masked), clear (all zero) or mixed (ship the transposed, pre-scaled
    tile). Deduplicates mixed tiles."""
    mq = np.asarray(mask).reshape(S, S)
    plan = []
    uniq = {}
    mlist = []
    for qi in range(NQT):
        row = []
        for kb in range(NQT):
            sub = mq[qi * 128:(qi + 1) * 128, kb * 128:(kb + 1) * 128]
            if np.all(sub <= -1e8):
                continue
            if np.all(sub == 0):
                row.append((kb, None))
                continue
            tt = np.ascontiguousarray(sub.T.astype(np.float32) / SCALE)
            key = tt.tobytes()
            if key not in uniq:
                uniq[key] = len(mlist)
                mlist.append(tt)
            row.append((kb, uniq[key]))
        plan.append(row)
    return plan, mlist


def _group_plan(plan, n_mtiles):
    """512-wide query groups. Per group: list of (kb, q0, q1, adds) with
    q0..q1 the covered query quarters and adds = [(quarter, mtile_idx)];
    mtile_idx == n_mtiles selects the -inf tile. The first kb of each
    group always spans the full group so PSUM has_written is set."""
    NEG = n_mtiles
    plan2 = []
    for qg in range(NQG):
        qmode = []
        for q in range(4):
            qmode.append(dict(plan[qg * 4 + q]))
        live = sorted(set().union(*[set(d.keys()) for d in qmode]))
        entries = []
        for j, kb in enumerate(live):
            pres = [kb in qmode[q] for q in range(4)]
            if j == 0:
                q0, q1 = 0, 3
            else:
                q0 = min(q for q in range(4) if pres[q])
                q1 = max(q for q in range(4) if pres[q])
            adds = []
            for q in range(q0, q1 + 1):
                if not pres[q]:
                    adds.append((q, NEG))
                elif qmode[q][kb] is not None:
                    adds.append((q, qmode[q][kb]))
            entries.append((kb, q0, q1, tuple(adds)))
        plan2.append(tuple(entries))
    return plan2


def _build_program(plan2, n_mt):
    """Build + compile the SPMD program (identical on all 8 cores).
    n_mt counts mask tiles INCLUDING the trailing -inf tile."""
    nc = bacc.Bacc("TRN2", target_bir_lowering=False, debug=False, num_devices=NC)

    # p-major host layouts so each logical group is ONE big DMA
    xt = nc.dram_tensor("xt", [4, 128, NDX, 512], BF16, kind="ExternalInput")
    wqt = nc.dram_tensor("wqt", [HLOC, 128, NDX, 128], BF16, kind="ExternalInput")
    wkt = nc.dram_tensor("wkt", [HLOC, 128, NDX, 128], BF16, kind="ExternalInput")
    wvt = nc.dram_tensor("wvt", [128, NDX, DLOC], BF16, kind="ExternalInput")
    wot = nc.dram_tensor("wot", [128, NDX, DLOC], BF16, kind="ExternalInput")
    pt = nc.dram_tensor("pt", [128, NDX, PL], BF16, kind="ExternalInput")
    cosT = nc.dram_tensor("cosT", [64, S], F32, kind="ExternalInput")
    sinT = nc.dram_tensor("sinT", [64, S], F32, kind="ExternalInput")
    gates = nc.dram_tensor("gates", [PL, HLOC], F32, kind="ExternalInput")
    mtiles = nc.dram_tensor("mtiles", [n_mt, 128, 128], BF16, kind="ExternalInput")
    ident = nc.dram_tensor("ident", [128, 128], BF16, kind="ExternalInput")
    out_d = nc.dram_tensor("out", [T, DLOC], F32, kind="ExternalOutput")

    AF = mybir.ActivationFunctionType
    OP = mybir.AluOpType
    labels = {}
    nc._unit_labels = labels

    def _lb(inst, tag):
        labels[inst.ins.name] = tag
        return inst

    with tile.TileContext(nc) as tc:
        with (
            tc.tile_pool(name="const", bufs=1) as cpool,
            tc.tile_pool(name="wres", bufs=1) as wres,
            tc.tile_pool(name="stream", bufs=1) as sp,
            tc.tile_pool(name="act", bufs=1) as ap,
            tc.tile_pool(name="psum", bufs=1, space="PSUM") as pp,
            tc.tile_pool(name="dram", bufs=1, space="DRAM") as dp,
        ):
            # ---- persistent constants / weights ----
            cos_sb = cpool.tile([64, S], F32, tag="cos")
            sin_sb = cpool.tile([64, S], F32, tag="sin")
            nc.gpsimd.dma_start(cos_sb[:], cosT[:])
            nc.gpsimd.dma_start(sin_sb[:], sinT[:])
            gates_sb = cpool.tile([PL, HLOC], F32, tag="gates")
            nc.gpsimd.dma_start(gates_sb[:], gates[:])
            mt_sb = []
            for i in range(n_mt):
                t = cpool.tile([128, 128], BF16, tag=f"mt{i}", name=f"mt{i}")
                nc.gpsimd.dma_start(t[:], mtiles[i])
                mt_sb.append(t)
            id_sb = cpool.tile([128, 128], BF16, tag="ident")
            nc.gpsimd.dma_start(id_sb[:], ident[:])
            ones_col = cpool.tile([128, 1], BF16, tag="ones_col")
            nc.vector.memset(ones_col[:], 1.0)

            # wv / wo resident: 4 tiles each of [128, 8*512]
            # (DMAs issued on the GpSimd queue after the first projection
            # group so they don't delay the critical-path x/wq loads)
            wv_sb = [wres.tile([128, 8 * DLOC], BF16, tag=f"wv{j}",
                               name=f"wv{j}") for j in range(4)]
            wo_sb = [wres.tile([128, 8 * DLOC], BF16, tag=f"wo{j}",
                               name=f"wo{j}") for j in range(4)]

            def emit_wvwo_loads():
                for j in range(4):
                    nc.gpsimd.dma_start(wv_sb[j][:], wvt[:, 8 * j:8 * (j + 1), :])
                for j in range(4):
                    nc.gpsimd.dma_start(wo_sb[j][:], wot[:, 8 * j:8 * (j + 1), :])

            def wv_sl(i):
                return wv_sb[i // 8][:, (i % 8) * DLOC:(i % 8 + 1) * DLOC]

            def wo_sl(i):
                return wo_sb[i // 8][:, (i % 8) * DLOC:(i % 8 + 1) * DLOC]

            pt_sb = cpool.tile([128, NDX * PL], BF16, tag="pt")
            nc.gpsimd.dma_start(pt_sb[:], pt[:])

            pk_sb = [ap.tile([128, PL], BF16, tag=f"pk{h}", name=f"pk{h}")
                     for h in range(HLOC)]
            pv_sb = ap.tile([PL, DLOC], BF16, tag="pv")

            # b=0: one AG per head; b=1 qg0: one AG per head-pair; b=1 qg1:
            # one AG per head so the tail collective after the last
            # attention unit is only 128KB and lands quickly.
            agin = {}
            agout = {}
            for h in range(HLOC):
                agin[0, h] = dp.tile([NQT, 128, 128], BF16,
                                     tag=f"agin0_{h}", name=f"agin0_{h}")
                agout[0, h] = dp.tile([NC, NQT, 128, 128], BF16,
                                      tag=f"agout0_{h}", name=f"agout0_{h}",
                                      addr_space="Shared")
            for hf in range(2):
                agin[1, 0, hf] = dp.tile([2, 4, 128, 128], BF16,
                                         tag=f"agin1_0_{hf}",
                                         name=f"agin1_0_{hf}")
                agout[1, 0, hf] = dp.tile([NC, 2, 4, 128, 128], BF16,
                                          tag=f"agout1_0_{hf}",
                                          name=f"agout1_0_{hf}",
                                          addr_space="Shared")
            aginh = {}
            agouth = {}
            for h in range(HLOC):
                aginh[h] = dp.tile([4, 128, 128], BF16, tag=f"aginh{h}",
                                   name=f"aginh{h}")
                agouth[h] = dp.tile([NC, 4, 128, 128], BF16, tag=f"agouth{h}",
                                    name=f"agouth{h}", addr_space="Shared")

            XT_BUFS = 4     # [128, 4096] quarters (one chunk live)
            WQK_BUFS = 2
            QK_BUFS = 6
            V_BUFS = NQT + 4
            AG_BUFS = 2

            qT = {}
            kT = {}
            v_sb = {}

            def gen_qkv(b, chunks=(0, 1)):
                for tc2 in chunks:
                    tcg = b * 2 + tc2
                    cols = slice(tc2 * 512, (tc2 + 1) * 512)
                    xts = [sp.tile([128, 8 * 512], BF16, tag="xt",
                                   bufs=XT_BUFS, name=f"xt{tcg}_{q}")
                           for q in range(4)]
                    xq0_eng = nc.sync if tcg == 0 else nc.scalar
                    xq0_eng.dma_start(xts[0][:, 0:2048], xt[tcg, :, 0:4, :])
                    xq0_eng.dma_start(xts[0][:, 2048:4096], xt[tcg, :, 4:8, :])
                    xlate = [(q, xts[q]) for q in range(1, 4)]

                    def x_sl(i):
                        return xts[i // 8][:, (i % 8) * 512:(i % 8 + 1) * 512]

                    if tc2 == 0:
                        qT[b] = [sp.tile([128, S], BF16, tag="qT", bufs=QK_BUFS,
                                         name=f"qT{b}_{j}") for j in range(HLOC)]
                        kT[b] = [sp.tile([128, S], BF16, tag="kT", bufs=QK_BUFS,
                                         name=f"kT{b}_{j}") for j in range(HLOC)]
                        v_sb[b] = [sp.tile([128, DLOC], BF16, tag="v", bufs=V_BUFS,
                                           name=f"v{b}_{j}") for j in range(NQT)]
                    # --- q & k projections (T-major out) + RoPE ---
                    for proj, wdram, dstT in ((0, wqt, qT[b]), (1, wkt, kT[b])):
                        for dqb in range(HLOC):
                            wt = sp.tile([128, NDX * 128], BF16, tag="wqk",
                                         bufs=WQK_BUFS)
                            nc.scalar.dma_start(wt[:, 0:2048],
                                                wdram[dqb, :, 0:16, :])
                            nc.scalar.dma_start(wt[:, 2048:4096],
                                                wdram[dqb, :, 16:32, :])
                            while xlate:
                                q, xtile = xlate.pop(0)
                                nc.scalar.dma_start(
                                    xtile[:], xt[tcg, :, 8 * q:8 * (q + 1), :])
                            ps = pp.tile([128, 512], F32, tag="mm512", bufs=2)
                            for i in range(NDX):
                                _lb(nc.tensor.matmul(
                                    ps[:], wt[:, i * 128:(i + 1) * 128], x_sl(i),
                                    start=(i == 0), stop=(i == NDX - 1)),
                                    f"qkv{b}.{tc2}.p{proj}.d{dqb}.{i}")
                            if proj == 1 and b == 0 and tc2 == 0:
                                # prompt keys for this head, reusing wk tiles
                                psk = pp.tile([128, 512], F32, tag="sc", bufs=2)
                                for i in range(NDX):
                                    nc.tensor.matmul(
                                        psk[:, 0:PL], wt[:, i * 128:(i + 1) * 128],
                                        pt_sb[:, i * PL:(i + 1) * PL],
                                        start=(i == 0), stop=(i == NDX - 1))
                                nc.vector.tensor_copy(pk_sb[dqb][:], psk[:, 0:PL])
                            # RoPE: rows 0:64 = even hd dims, 64:128 = odd
                            c_sl = cos_sb[:, cols]
                            s_sl = sin_sb[:, cols]
                            t_rc = sp.tile([64, 512], BF16, tag="rt", bufs=4)
                            t_rs = sp.tile([64, 512], BF16, tag="rt", bufs=4)
                            t_ic = sp.tile([64, 512], BF16, tag="rt", bufs=4)
                            t_is = sp.tile([64, 512], BF16, tag="rt", bufs=4)
                            nc.vector.tensor_tensor(t_rc[:], ps[0:64, :], c_sl, op=OP.mult)
                            nc.vector.tensor_tensor(t_rs[:], ps[0:64, :], s_sl, op=OP.mult)
                            nc.vector.tensor_tensor(t_ic[:], ps[64:128, :], c_sl, op=OP.mult)
                            nc.vector.tensor_tensor(t_is[:], ps[64:128, :], s_sl, op=OP.mult)
                            nc.vector.tensor_tensor(dstT[dqb][0:64, cols], t_rc[:],
                                                    t_is[:], op=OP.subtract)
                            nc.gpsimd.tensor_tensor(dstT[dqb][64:128, cols], t_rs[:],
                                                    t_ic[:], op=OP.add)
                            yield
                    # --- v projection (natural [t, dv]) ---
                    for tblk in range(4):
                        ps = pp.tile([128, 512], F32, tag="mm512", bufs=2)
                        for i in range(NDX):
                            _lb(nc.tensor.matmul(
                                ps[:], x_sl(i)[:, tblk * 128:(tblk + 1) * 128],
                                wv_sl(i), start=(i == 0), stop=(i == NDX - 1)),
                                f"v{b}.{tc2}.{tblk}.{i}")
                        nc.vector.tensor_copy(v_sb[b][tc2 * 4 + tblk][:], ps[:])
                        yield
                    if b == 0 and tc2 == 0:
                        psv = pp.tile([128, 512], F32, tag="mm512", bufs=2)
                        for i in range(NDX):
                            nc.tensor.matmul(psv[0:PL, :],
                                             pt_sb[:, i * PL:(i + 1) * PL],
                                             wv_sl(i),
                                             start=(i == 0), stop=(i == NDX - 1))
                        nc.vector.tensor_copy(pv_sb[:], psv[0:PL, :])
                        for hh in range(HLOC):
                            nc.vector.tensor_scalar(
                                pv_sb[0:PL, hh * 128:(hh + 1) * 128],
                                pv_sb[0:PL, hh * 128:(hh + 1) * 128],
                                gates_sb[0:PL, hh:hh + 1], None, op0=OP.mult)

            def gen_att(b):
                if b == 0:
                    hq_order = [(h, qg) for h in range(HLOC) for qg in range(NQG)]
                else:
                    # qg-outer so every head's qg0 AllGather lands mid-phase
                    hq_order = [(h, qg) for qg in range(NQG) for h in range(HLOC)]
                for h, qg in hq_order:
                    if True:
                        stage = sp.tile([128, 512], BF16, tag="stage", bufs=2,
                                        name=f"stage{b}_{h}_{qg}")
                        qbase = qg * 512
                        entries = plan2[qg]
                        probs = []
                        for kb, q0, q1, adds in entries:
                            coff = q0 * 128
                            ncols = (q1 - q0 + 1) * 128
                            ssc = pp.tile([128, 512], F32, tag="sc", bufs=2)
                            _lb(nc.tensor.matmul(
                                ssc[:, coff:coff + ncols],
                                kT[b][h][:, kb * 128:(kb + 1) * 128],
                                qT[b][h][:, qbase + coff:qbase + coff + ncols],
                                start=True, stop=(not adds)),
                                f"sc{b}.h{h}.g{qg}.k{kb}")
                            for ai, (q, idx) in enumerate(adds):
                                nc.tensor.matmul(
                                    ssc[:, q * 128:(q + 1) * 128], id_sb[:],
                                    mt_sb[idx][:], start=False,
                                    stop=(ai == len(adds) - 1))
                            pr = sp.tile([128, 512], BF16, tag="probs", bufs=9)
                            nc.scalar.activation(pr[:, coff:coff + ncols],
                                                 ssc[:, coff:coff + ncols],
                                                 AF.Exp, scale=SCALE)
                            probs.append((kb, coff, ncols, pr))
                        # prompt scores
                        psc = pp.tile([128, 512], F32, tag="sc", bufs=2)
                        nc.tensor.matmul(psc[0:PL, :], pk_sb[h][:],
                                         qT[b][h][:, qbase:qbase + 512],
                                         start=True, stop=True)
                        ppr = sp.tile([PL, 512], BF16, tag="pprobs", bufs=1)
                        nc.scalar.activation(ppr[:], psc[0:PL, :], AF.Exp,
                                             scale=SCALE)
                        # PV accumulation + sums
                        po = pp.tile([128, 512], F32, tag="pv", bufs=3)
                        pss = pp.tile([128, 512], F32, tag="aux", bufs=1)
                        n = len(probs)
                        for i, (kb, coff, ncols, pr) in enumerate(probs):
                            _lb(nc.tensor.matmul(
                                po[:, coff:coff + ncols],
                                v_sb[b][kb][:, h * 128:(h + 1) * 128],
                                pr[:, coff:coff + ncols],
                                start=(i == 0), stop=(i == n - 1)),
                                f"pv{b}.h{h}.g{qg}.k{kb}")
                        for i, (kb, coff, ncols, pr) in enumerate(probs):
                            nc.tensor.matmul(
                                pss[0:1, coff:coff + ncols], ones_col[:, 0:1],
                                pr[:, coff:coff + ncols],
                                start=(i == 0), stop=(i == n - 1))
                        ppo = pp.tile([128, 512], F32, tag="pv", bufs=3)
                        nc.tensor.matmul(ppo[:], pv_sb[0:PL, h * 128:(h + 1) * 128],
                                         ppr[:], start=True, stop=True)
                        nc.tensor.matmul(pss[32:33, :], ones_col[0:PL, 0:1], ppr[:],
                                         start=True, stop=True)
                        # 1/s = exp(-ln(s)) on the (otherwise idle) ScalarE
                        lt = sp.tile([1, 1024], F32, tag="lnt", bufs=1)
                        nc.scalar.activation(lt[0:1, 0:512], pss[0:1, :], AF.Ln)
                        nc.scalar.activation(lt[0:1, 512:1024], pss[32:33, :], AF.Ln)
                        recs = sp.tile([1, 1024], BF16, tag="recs", bufs=1)
                        nc.scalar.activation(recs[:], lt[:], AF.Exp, scale=-1.0)
                        # broadcast row-vector across partitions (GpSimd)
                        bcs = sp.tile([128, 1024], BF16, tag="bcs", bufs=2)
                        nc.gpsimd.partition_broadcast(bcs[:], recs[0:1, :])
                        po_c = sp.tile([128, 512], BF16, tag="poc", bufs=2)
                        ppo_c = sp.tile([128, 512], BF16, tag="poc", bufs=2)
                        nc.vector.tensor_copy(po_c[:], po[:])
                        nc.vector.tensor_copy(ppo_c[:], ppo[:])
                        t1 = sp.tile([128, 512], BF16, tag="cmb", bufs=2)
                        t2 = sp.tile([128, 512], BF16, tag="cmb", bufs=2)
                        nc.gpsimd.tensor_tensor(t1[:], po_c[:], bcs[:, 0:512],
                                                op=OP.mult)
                        nc.gpsimd.tensor_tensor(t2[:], ppo_c[:], bcs[:, 512:1024],
                                                op=OP.mult)
                        nc.gpsimd.tensor_tensor(stage[:], t1[:], t2[:], op=OP.add)
                        if b == 0:
                            nc.sync.dma_start(
                                agin[0, h][qg * 4:(qg + 1) * 4].rearrange(
                                    "n p c -> p n c"), stage[:])
                            if qg == NQG - 1:
                                nc.gpsimd.collective_compute(
                                    "AllGather", OP.bypass,
                                    replica_groups=[list(range(NC))],
                                    ins=[agin[0, h].opt()],
                                    outs=[agout[0, h].opt()])
                        elif qg == 0:
                            nc.sync.dma_start(
                                agin[1, 0, h // 2][h % 2].rearrange(
                                    "n p c -> p n c"), stage[:])
                            if h % 2 == 1:
                                nc.gpsimd.collective_compute(
                                    "AllGather", OP.bypass,
                                    replica_groups=[list(range(NC))],
                                    ins=[agin[1, 0, h // 2].opt()],
                                    outs=[agout[1, 0, h // 2].opt()])
                        else:
                            nc.sync.dma_start(
                                aginh[h].rearrange("n p c -> p n c"), stage[:])
                            nc.gpsimd.collective_compute(
                                "AllGather", OP.bypass,
                                replica_groups=[list(range(NC))],
                                ins=[aginh[h].opt()],
                                outs=[agouth[h].opt()])
                        yield

            def _agt_fetch(b, tq):
                agt = sp.tile([128, NDX * 128], BF16, tag="ag",
                              bufs=AG_BUFS, name=f"ag{b}_{tq}")
                agt_v = agt[:].rearrange("p (n h c) -> p n h c",
                                         n=NC, h=HLOC, c=128)
                for hl in range(HLOC):
                    if b == 0:
                        src = agout[0, hl][:, tq]
                    elif tq < 4:
                        src = agout[1, 0, hl // 2][:, hl % 2, tq % 4]
                    else:
                        src = agouth[hl][:, tq % 4]
                    eng = nc.sync if hl % 2 == 0 else nc.scalar
                    eng.dma_start(agt_v[:, :, hl, :],
                                  src.rearrange("n p c -> p n c"))
                return agt

            def gen_oproj(b, tqs):
                for tq in tqs:
                    agt = _agt_fetch(b, tq)
                    pso = pp.tile([128, 512], F32, tag="mm512", bufs=2)
                    order = [core * HLOC + hl for hl in range(HLOC)
                             for core in range(NC)]
                    for j, i in enumerate(order):
                        _lb(nc.tensor.matmul(
                            pso[:], agt[:, i * 128:(i + 1) * 128],
                            wo_sl(i), start=(j == 0), stop=(j == NDX - 1)),
                            f"op{b}.t{tq}.{i}")
                    ost = sp.tile([128, 512], F32, tag="ost", bufs=1)
                    nc.vector.tensor_copy(ost[:], pso[:])
                    r0 = b * S + tq * 128
                    nc.sync.dma_start(out_d[r0:r0 + 128, :], ost[:])
                    yield

            # ---- software-pipelined emission ----
            g_qkv0 = gen_qkv(0)
            next(g_qkv0)
            emit_wvwo_loads()
            for _ in g_qkv0:
                pass
            g_att0, g_qkv1 = gen_att(0), gen_qkv(1)
            for _ in g_att0:
                for _ in range(3):
                    next(g_qkv1, None)
            for _ in g_qkv1:
                pass
            # att1: interleave most of o0; hold back the rest of o0 plus all
            # of o1 for the tail, which is then pure PE work that covers the
            # last collectives' wire time.
            g_att1 = gen_att(1)
            g_o0 = gen_oproj(0, list(range(NQT)))
            g_o1 = gen_oproj(1, list(range(NQT)))
            cnt = 0
            for _ in g_att1:
                cnt += 1
                if cnt >= 3:
                    next(g_o0, None)
                    next(g_o0, None)
            for _ in g_o0:
                pass
            for _ in g_o1:
                pass

    nc.compile()
    return nc


def kernel(**inputs):
    x = np.asarray(inputs["x"], np.float32)
    wq = np.asarray(inputs["wq"], np.float32)
    wk = np.asarray(inputs["wk"], np.float32)
    wv = np.asarray(inputs["wv"], np.float32)
    wo = np.asarray(inputs["wo"], np.float32)
    prompt = np.asarray(inputs["prompt"], np.float32)
    prompt_gate = np.asarray(inputs["prompt_gate"], np.float32)
    freqs_cos = np.asarray(inputs["freqs_cos"], np.float32)
    freqs_sin = np.asarray(inputs["freqs_sin"], np.float32)
    mask = np.asarray(inputs["mask"], np.float32)

    plan, mlist = _analyze_mask(mask)
    plan2 = _group_plan(plan, len(mlist))
    n_mt = len(mlist) + 1  # + trailing -inf tile
    plan_key = (tuple(plan2), n_mt)
    if plan_key not in _PROG_CACHE:
        _PROG_CACHE[plan_key] = _build_program(plan2, n_mt)
    nc = _PROG_CACHE[plan_key]

    # ---- shared host prep ----
    perm = np.concatenate([np.arange(0, HD, 2), np.arange(1, HD, 2)])
    xT = np.ascontiguousarray(x.reshape(T, D).T.astype(NPBF16))
    # [4, 128, NDX, 512]: [tcg, dx_in_block, dx_block, t_in_chunk]
    xt_tiles = np.ascontiguousarray(
        xT.reshape(NDX, 128, 4, 512).transpose(2, 1, 0, 3))
    ptT = np.ascontiguousarray(prompt.T.astype(NPBF16))       # [D, PL]
    pt_tiles = np.ascontiguousarray(
        ptT.reshape(NDX, 128, PL).transpose(1, 0, 2))
    cosT = np.ascontiguousarray(freqs_cos.T.astype(np.float32))
    sinT = np.ascontiguousarray(freqs_sin.T.astype(np.float32))
    neg = np.full((1, 128, 128), -1e30, np.float32)
    if mlist:
        mtiles = np.concatenate([np.stack(mlist), neg]).astype(NPBF16)
    else:
        mtiles = neg.astype(NPBF16)

    def shard_qk(w, c):
        rows = np.concatenate(
            [c * DLOC + j * HD + perm for j in range(HLOC)])
        wT = w[rows, :].T.astype(NPBF16)                      # [D, DLOC]
        return np.ascontiguousarray(
            wT.reshape(NDX, 128, HLOC, 128).transpose(2, 1, 0, 3))

    def shard_rhs(w, c):
        # rows c*DLOC..+DLOC of w, transposed -> [D, DLOC] -> [128,NDX,DLOC]
        wT = w[c * DLOC:(c + 1) * DLOC, :].T.astype(NPBF16)
        return np.ascontiguousarray(wT.reshape(NDX, 128, DLOC).transpose(1, 0, 2))

    in_maps = []
    for c in range(NC):
        in_maps.append(dict(
            xt=xt_tiles,
            wqt=shard_qk(wq, c),
            wkt=shard_qk(wk, c),
            wvt=shard_rhs(wv, c),
            wot=shard_rhs(wo, c),
            pt=pt_tiles,
            cosT=cosT,
            sinT=sinT,
            gates=np.ascontiguousarray(np.repeat(
                prompt_gate.reshape(H)[c * HLOC:(c + 1) * HLOC][None, :],
                PL, axis=0)).astype(np.float32),
            mtiles=mtiles,
            ident=np.eye(128, dtype=NPBF16),
        ))

    res = bass_utils.run_bass_kernel_spmd(
        nc, in_maps, core_ids=list(range(NC)),
        trace=bool(os.environ.get("BASS_TRACE")))
    kernel.last_result = res

    full = np.empty((T, D), np.float32)
    for c in range(NC):
        full[:, c * DLOC:(c + 1) * DLOC] = res.results[c]["out"]
    return full.reshape(B, S, D)


# revision 22
# speedup vs baseline: 1.0896x; 1.0372x over previous
"""Trainium2 Bass kernel: multi-head attention with RoPE + gated prompt
injection (nn_Attention_28080496181816), sharded over 8 NeuronCores.

Sharding: tensor-parallel over heads. Core c owns heads [4c, 4c+4):
  - wq/wk/wv column-sharded (per-head), o-proj via AllGather of the
    per-core attention outputs + column-sharded wo matmul.
  - Host-side unshard is a pure concatenation of output column slices.

Layout: "T-major" — activations live as [feature, token] on device so
every matmul contraction lands on the partition axis with no on-device
transposes. RoPE pairs are made contiguous by permuting wq/wk rows
(per head: even hd dims then odd hd dims) on the host.

Attention runs in 512-wide query groups with variable-width score
matmuls derived from the runtime mask structure (causal -> exact
lower-triangle work). The emission order software-pipelines phases so
the in-order PE stream interleaves attention with the next batch's
projections and the previous batch's output projection.
"""

import math
import os
import sys
import types

import numpy as np
import ml_dtypes

# --- optional NTFF profile hook shim (only needed if BASS_TRACE is set;
# the stock image lacks antenv.axon_hooks) ---
try:
    import antenv.axon_hooks  # noqa: F401
except Exception:
    try:
        import antenv
        _m = types.ModuleType("antenv.axon_hooks")
        _hook = [None]
        _m.set_axon_ntff_profile_hook = lambda h: _hook.__setitem__(0, h)
        _m.get_axon_ntff_profile_hook = lambda: _hook[0]
        sys.modules["antenv.axon_hooks"] = _m
        antenv.axon_hooks = _m
        from trn_agent_boot.trn_boot import _ntff_profile_via_ctypes
        _p = _ntff_profile_via_ctypes("/opt/axon/libaxon_pjrt.so")
        if _p is not None:
            _m.set_axon_ntff_profile_hook(_p)
    except Exception:
        pass

import concourse.bacc as bacc
import concourse.mybir as mybir
import concourse.tile as tile
from concourse import bass_utils

# If tracing is enabled in an environment without an artifact bucket,
# don't let the upload step crash the run.
_orig_upload = bass_utils.upload_artifacts


def _safe_upload(tmpdir):
    try:
        return _orig_upload(tmpdir)
    except Exception:
        return tmpdir


bass_utils.upload_artifacts = _safe_upload

BF16 = mybir.dt.bfloat16
F32 = mybir.dt.float32
NPBF16 = ml_dtypes.bfloat16

B, S, D, H, HD, PL = 2, 1024, 4096, 32, 128, 10
NC = 8              # cores
HLOC = H // NC      # 4 heads per core
DLOC = HLOC * HD    # 512
T = B * S           # 2048
NDX = D // 128      # 32 contraction blocks
NQT = S // 128      # 8 query tiles per batch
NQG = NQT // 4      # 2 query groups of 512
SCALE = 1.0 / math.sqrt(HD)

_PROG_CACHE = {}


def _analyze_mask(mask):
    """Classify each 128x128 tile of the additive mask: skip (fully
    masked), clear (all zero) or mixed (ship the transposed, pre-scaled
    tile). Deduplicates mixed tiles."""
    mq = np.asarray(mask).reshape(S, S)
    plan = []
    uniq = {}
    mlist = []
    for qi in range(NQT):
        row = []
        for kb in range(NQT):
            sub = mq[qi * 128:(qi + 1) * 128, kb * 128:(kb + 1) * 128]
            if np.all(sub <= -1e8):
                continue
            if np.all(sub == 0):
                row.append((kb, None))
                continue
            tt = np.ascontiguousarray(sub.T.astype(np.float32) / SCALE)
            key = tt.tobytes()
            if key not in uniq:
                uniq[key] = len(mlist)
                mlist.append(tt)
            row.append((kb, uniq[key]))
        plan.append(row)
    return plan, mlist


def _group_plan(plan, n_mtiles):
    """512-wide query groups. Per group: list of (kb, q0, q1, adds) with
    q0..q1 the covered query quarters and adds = [(quarter, mtile_idx)];
    mtile_idx == n_mtiles selects the -inf tile. The first kb of each
    group always spans the full group so PSUM has_written is set."""
    NEG = n_mtiles
    plan2 = []
    for qg in range(NQG):
        qmode = []
        for q in range(4):
            qmode.append(dict(plan[qg * 4 + q]))
        live = sorted(set().union(*[set(d.keys()) for d in qmode]))
        entries = []
        for j, kb in enumerate(live):
            pres = [kb in qmode[q] for q in range(4)]
            if j == 0:
                q0, q1 = 0, 3
            else:
                q0 = min(q for q in range(4) if pres[q])
                q1 = max(q for q in range(4) if pres[q])
            adds = []
            for q in range(q0, q1 + 1):
                if not pres[q]:
                    adds.append((q, NEG))
                elif qmode[q][kb] is not None:
                    adds.append((q, qmode[q][kb]))
            entries.append((kb, q0, q1, tuple(adds)))
        plan2.append(tuple(entries))
    return plan2


def _build_program(plan2, n_mt):
    """Build + compile the SPMD program (identical on all 8 cores).
    n_mt counts mask tiles INCLUDING the trailing -inf tile."""
    nc = bacc.Bacc("TRN2", target_bir_lowering=False, debug=False, num_devices=NC)

    # p-major host layouts so each logical group is ONE big DMA
    xt = nc.dram_tensor("xt", [4, 128, NDX, 512], BF16, kind="ExternalInput")
    wqt = nc.dram_tensor("wqt", [HLOC, 128, NDX, 128], BF16, kind="ExternalInput")
    wkt = nc.dram_tensor("wkt", [HLOC, 128, NDX, 128], BF16, kind="ExternalInput")
    wvt = nc.dram_tensor("wvt", [128, NDX, DLOC], BF16, kind="ExternalInput")
    wot = nc.dram_tensor("wot", [128, NDX, DLOC], BF16, kind="ExternalInput")
    pt = nc.dram_tensor("pt", [128, NDX, PL], BF16, kind="ExternalInput")
    cosT = nc.dram_tensor("cosT", [64, S], F32, kind="ExternalInput")
    sinT = nc.dram_tensor("sinT", [64, S], F32, kind="ExternalInput")
    gates = nc.dram_tensor("gates", [PL, HLOC], F32, kind="ExternalInput")
    mtiles = nc.dram_tensor("mtiles", [n_mt, 128, 128], BF16, kind="ExternalInput")
    ident = nc.dram_tensor("ident", [128, 128], BF16, kind="ExternalInput")
    out_d = nc.dram_tensor("out", [T, DLOC], F32, kind="ExternalOutput")

    AF = mybir.ActivationFunctionType
    OP = mybir.AluOpType
    labels = {}
    nc._unit_labels = labels

    def _lb(inst, tag):
        labels[inst.ins.name] = tag
        return inst

    with tile.TileContext(nc) as tc:
        with (
            tc.tile_pool(name="const", bufs=1) as cpool,
            tc.tile_pool(name="wres", bufs=1) as wres,
            tc.tile_pool(name="stream", bufs=1) as sp,
            tc.tile_pool(name="act", bufs=1) as ap,
            tc.tile_pool(name="psum", bufs=1, space="PSUM") as pp,
            tc.tile_pool(name="dram", bufs=1, space="DRAM") as dp,
        ):
            # ---- persistent constants / weights ----
            cos_sb = cpool.tile([64, S], F32, tag="cos")
            sin_sb = cpool.tile([64, S], F32, tag="sin")
            nc.gpsimd.dma_start(cos_sb[:], cosT[:])
            nc.gpsimd.dma_start(sin_sb[:], sinT[:])
            gates_sb = cpool.tile([PL, HLOC], F32, tag="gates")
            nc.gpsimd.dma_start(gates_sb[:], gates[:])
            mt_sb = []
            for i in range(n_mt):
                t = cpool.tile([128, 128], BF16, tag=f"mt{i}", name=f"mt{i}")
                nc.gpsimd.dma_start(t[:], mtiles[i])
                mt_sb.append(t)
            id_sb = cpool.tile([128, 128], BF16, tag="ident")
            nc.gpsimd.dma_start(id_sb[:], ident[:])
            ones_col = cpool.tile([128, 1], BF16, tag="ones_col")
            nc.vector.memset(ones_col[:], 1.0)

            # wv / wo resident: 4 tiles each of [128, 8*512]
            # (DMAs issued on the GpSimd queue after the first projection
            # group so they don't delay the critical-path x/wq loads)
            wv_sb = [wres.tile([128, 8 * DLOC], BF16, tag=f"wv{j}",
                               name=f"wv{j}") for j in range(4)]
            wo_sb = [wres.tile([128, 8 * DLOC], BF16, tag=f"wo{j}",
                               name=f"wo{j}") for j in range(4)]

            def emit_wvwo_loads():
                for j in range(4):
                    nc.gpsimd.dma_start(wv_sb[j][:], wvt[:, 8 * j:8 * (j + 1), :])
                for j in range(4):
                    nc.gpsimd.dma_start(wo_sb[j][:], wot[:, 8 * j:8 * (j + 1), :])

            def wv_sl(i):
                return wv_sb[i // 8][:, (i % 8) * DLOC:(i % 8 + 1) * DLOC]

            def wo_sl(i):
                return wo_sb[i // 8][:, (i % 8) * DLOC:(i % 8 + 1) * DLOC]

            pt_sb = cpool.tile([128, NDX * PL], BF16, tag="pt")
            nc.gpsimd.dma_start(pt_sb[:], pt[:])

            pk_sb = [ap.tile([128, PL], BF16, tag=f"pk{h}", name=f"pk{h}")
                     for h in range(HLOC)]
            pv_sb = ap.tile([PL, DLOC], BF16, tag="pv")

            # b=0: one AG per head; b=1 qg0: one AG per head-pair; b=1 qg1:
            # one AG per head so the tail collective after the last
            # attention unit is only 128KB and lands quickly.
            agin = {}
            agout = {}
            for h in range(HLOC):
                agin[0, h] = dp.tile([NQT, 128, 128], BF16,
                                     tag=f"agin0_{h}", name=f"agin0_{h}")
                agout[0, h] = dp.tile([NC, NQT, 128, 128], BF16,
                                      tag=f"agout0_{h}", name=f"agout0_{h}",
                                      addr_space="Shared")
            for qg in range(NQG):
                for hf in range(2):
                    agin[1, qg, hf] = dp.tile([2, 4, 128, 128], BF16,
                                              tag=f"agin1_{qg}_{hf}",
                                              name=f"agin1_{qg}_{hf}")
                    agout[1, qg, hf] = dp.tile([NC, 2, 4, 128, 128], BF16,
                                               tag=f"agout1_{qg}_{hf}",
                                               name=f"agout1_{qg}_{hf}",
                                               addr_space="Shared")

            XT_BUFS = 4     # [128, 4096] quarters (one chunk live)
            WQK_BUFS = 2
            QK_BUFS = 6
            V_BUFS = NQT + 4
            AG_BUFS = 2

            qT = {}
            kT = {}
            v_sb = {}

            def gen_qkv(b, chunks=(0, 1)):
                for tc2 in chunks:
                    tcg = b * 2 + tc2
                    cols = slice(tc2 * 512, (tc2 + 1) * 512)
                    xts = [sp.tile([128, 8 * 512], BF16, tag="xt",
                                   bufs=XT_BUFS, name=f"xt{tcg}_{q}")
                           for q in range(4)]
                    xq0_eng = nc.sync if tcg == 0 else nc.scalar
                    xq0_eng.dma_start(xts[0][:, 0:2048], xt[tcg, :, 0:4, :])
                    xq0_eng.dma_start(xts[0][:, 2048:4096], xt[tcg, :, 4:8, :])
                    xlate = [(q, xts[q]) for q in range(1, 4)]

                    def x_sl(i):
                        return xts[i // 8][:, (i % 8) * 512:(i % 8 + 1) * 512]

                    if tc2 == 0:
                        qT[b] = [sp.tile([128, S], BF16, tag="qT", bufs=QK_BUFS,
                                         name=f"qT{b}_{j}") for j in range(HLOC)]
                        kT[b] = [sp.tile([128, S], BF16, tag="kT", bufs=QK_BUFS,
                                         name=f"kT{b}_{j}") for j in range(HLOC)]
                        v_sb[b] = [sp.tile([128, DLOC], BF16, tag="v", bufs=V_BUFS,
                                           name=f"v{b}_{j}") for j in range(NQT)]
                    # --- q & k projections (T-major out) + RoPE ---
                    for proj, wdram, dstT in ((0, wqt, qT[b]), (1, wkt, kT[b])):
                        for dqb in range(HLOC):
                            wt = sp.tile([128, NDX * 128], BF16, tag="wqk",
                                         bufs=WQK_BUFS)
                            nc.scalar.dma_start(wt[:, 0:2048],
                                                wdram[dqb, :, 0:16, :])
                            nc.scalar.dma_start(wt[:, 2048:4096],
                                                wdram[dqb, :, 16:32, :])
                            while xlate:
                                q, xtile = xlate.pop(0)
                                nc.scalar.dma_start(
                                    xtile[:], xt[tcg, :, 8 * q:8 * (q + 1), :])
                            ps = pp.tile([128, 512], F32, tag="mm512", bufs=2)
                            for i in range(NDX):
                                _lb(nc.tensor.matmul(
                                    ps[:], wt[:, i * 128:(i + 1) * 128], x_sl(i),
                                    start=(i == 0), stop=(i == NDX - 1)),
                                    f"qkv{b}.{tc2}.p{proj}.d{dqb}.{i}")
                            if proj == 1 and b == 0 and tc2 == 0:
                                # prompt keys for this head, reusing wk tiles
                                psk = pp.tile([128, 512], F32, tag="sc", bufs=2)
                                for i in range(NDX):
                                    nc.tensor.matmul(
                                        psk[:, 0:PL], wt[:, i * 128:(i + 1) * 128],
                                        pt_sb[:, i * PL:(i + 1) * PL],
                                        start=(i == 0), stop=(i == NDX - 1))
                                nc.vector.tensor_copy(pk_sb[dqb][:], psk[:, 0:PL])
                            # RoPE: rows 0:64 = even hd dims, 64:128 = odd
                            c_sl = cos_sb[:, cols]
                            s_sl = sin_sb[:, cols]
                            t_rc = sp.tile([64, 512], BF16, tag="rt", bufs=4)
                            t_rs = sp.tile([64, 512], BF16, tag="rt", bufs=4)
                            t_ic = sp.tile([64, 512], BF16, tag="rt", bufs=4)
                            t_is = sp.tile([64, 512], BF16, tag="rt", bufs=4)
                            nc.vector.tensor_tensor(t_rc[:], ps[0:64, :], c_sl, op=OP.mult)
                            nc.vector.tensor_tensor(t_rs[:], ps[0:64, :], s_sl, op=OP.mult)
                            nc.vector.tensor_tensor(t_ic[:], ps[64:128, :], c_sl, op=OP.mult)
                            nc.vector.tensor_tensor(t_is[:], ps[64:128, :], s_sl, op=OP.mult)
                            nc.vector.tensor_tensor(dstT[dqb][0:64, cols], t_rc[:],
                                                    t_is[:], op=OP.subtract)
                            nc.gpsimd.tensor_tensor(dstT[dqb][64:128, cols], t_rs[:],
                                                    t_ic[:], op=OP.add)
                            yield
                    # --- v projection (natural [t, dv]) ---
                    for tblk in range(4):
                        ps = pp.tile([128, 512], F32, tag="mm512", bufs=2)
                        for i in range(NDX):
                            _lb(nc.tensor.matmul(
                                ps[:], x_sl(i)[:, tblk * 128:(tblk + 1) * 128],
                                wv_sl(i), start=(i == 0), stop=(i == NDX - 1)),
                                f"v{b}.{tc2}.{tblk}.{i}")
                        nc.vector.tensor_copy(v_sb[b][tc2 * 4 + tblk][:], ps[:])
                        yield
                    if b == 0 and tc2 == 0:
                        psv = pp.tile([128, 512], F32, tag="mm512", bufs=2)
                        for i in range(NDX):
                            nc.tensor.matmul(psv[0:PL, :],
                                             pt_sb[:, i * PL:(i + 1) * PL],
                                             wv_sl(i),
                                             start=(i == 0), stop=(i == NDX - 1))
                        nc.vector.tensor_copy(pv_sb[:], psv[0:PL, :])
                        for hh in range(HLOC):
                            nc.vector.tensor_scalar(
                                pv_sb[0:PL, hh * 128:(hh + 1) * 128],
                                pv_sb[0:PL, hh * 128:(hh + 1) * 128],
                                gates_sb[0:PL, hh:hh + 1], None, op0=OP.mult)

            def gen_att(b):
                if b == 0:
                    hq_order = [(h, qg) for h in range(HLOC) for qg in range(NQG)]
                else:
                    # qg-outer so every head's qg0 AllGather lands mid-phase
                    hq_order = [(h, qg) for qg in range(NQG) for h in range(HLOC)]
                for h, qg in hq_order:
                    if True:
                        stage = sp.tile([128, 512], BF16, tag="stage", bufs=2,
                                        name=f"stage{b}_{h}_{qg}")
                        qbase = qg * 512
                        entries = plan2[qg]
                        probs = []
                        for kb, q0, q1, adds in entries:
                            coff = q0 * 128
                            ncols = (q1 - q0 + 1) * 128
                            ssc = pp.tile([128, 512], F32, tag="sc", bufs=2)
                            _lb(nc.tensor.matmul(
                                ssc[:, coff:coff + ncols],
                                kT[b][h][:, kb * 128:(kb + 1) * 128],
                                qT[b][h][:, qbase + coff:qbase + coff + ncols],
                                start=True, stop=(not adds)),
                                f"sc{b}.h{h}.g{qg}.k{kb}")
                            for ai, (q, idx) in enumerate(adds):
                                nc.tensor.matmul(
                                    ssc[:, q * 128:(q + 1) * 128], id_sb[:],
                                    mt_sb[idx][:], start=False,
                                    stop=(ai == len(adds) - 1))
                            pr = sp.tile([128, 512], BF16, tag="probs", bufs=9)
                            nc.scalar.activation(pr[:, coff:coff + ncols],
                                                 ssc[:, coff:coff + ncols],
                                                 AF.Exp, scale=SCALE)
                            probs.append((kb, coff, ncols, pr))
                        # prompt scores
                        psc = pp.tile([128, 512], F32, tag="sc", bufs=2)
                        nc.tensor.matmul(psc[0:PL, :], pk_sb[h][:],
                                         qT[b][h][:, qbase:qbase + 512],
                                         start=True, stop=True)
                        ppr = sp.tile([PL, 512], BF16, tag="pprobs", bufs=1)
                        nc.scalar.activation(ppr[:], psc[0:PL, :], AF.Exp,
                                             scale=SCALE)
                        # PV accumulation + sums
                        po = pp.tile([128, 512], F32, tag="pv", bufs=3)
                        pss = pp.tile([128, 512], F32, tag="aux", bufs=1)
                        n = len(probs)
                        for i, (kb, coff, ncols, pr) in enumerate(probs):
                            _lb(nc.tensor.matmul(
                                po[:, coff:coff + ncols],
                                v_sb[b][kb][:, h * 128:(h + 1) * 128],
                                pr[:, coff:coff + ncols],
                                start=(i == 0), stop=(i == n - 1)),
                                f"pv{b}.h{h}.g{qg}.k{kb}")
                        for i, (kb, coff, ncols, pr) in enumerate(probs):
                            nc.tensor.matmul(
                                pss[0:1, coff:coff + ncols], ones_col[:, 0:1],
                                pr[:, coff:coff + ncols],
                                start=(i == 0), stop=(i == n - 1))
                        ppo = pp.tile([128, 512], F32, tag="pv", bufs=3)
                        nc.tensor.matmul(ppo[:], pv_sb[0:PL, h * 128:(h + 1) * 128],
                                         ppr[:], start=True, stop=True)
                        nc.tensor.matmul(pss[32:33, :], ones_col[0:PL, 0:1], ppr[:],
                                         start=True, stop=True)
                        # 1/s on the Vector engine (frees ScalarE for exp)
                        rec4 = sp.tile([1, 1024], F32, tag="lnt", bufs=1)
                        nc.vector.reciprocal(rec4[0:1, 0:512], pss[0:1, :])
                        nc.vector.reciprocal(rec4[0:1, 512:1024], pss[32:33, :])
                        recs = sp.tile([1, 1024], BF16, tag="recs", bufs=1)
                        nc.vector.tensor_copy(recs[:], rec4[:])
                        # broadcast row-vector across partitions (GpSimd)
                        bcs = sp.tile([128, 1024], BF16, tag="bcs", bufs=2)
                        nc.gpsimd.partition_broadcast(bcs[:], recs[0:1, :])
                        po_c = sp.tile([128, 512], BF16, tag="poc", bufs=2)
                        ppo_c = sp.tile([128, 512], BF16, tag="poc", bufs=2)
                        nc.vector.tensor_copy(po_c[:], po[:])
                        nc.vector.tensor_copy(ppo_c[:], ppo[:])
                        t1 = sp.tile([128, 512], BF16, tag="cmb", bufs=2)
                        t2 = sp.tile([128, 512], BF16, tag="cmb", bufs=2)
                        nc.gpsimd.tensor_tensor(t1[:], po_c[:], bcs[:, 0:512],
                                                op=OP.mult)
                        nc.gpsimd.tensor_tensor(t2[:], ppo_c[:], bcs[:, 512:1024],
                                                op=OP.mult)
                        nc.gpsimd.tensor_tensor(stage[:], t1[:], t2[:], op=OP.add)
                        if b == 0:
                            nc.sync.dma_start(
                                agin[0, h][qg * 4:(qg + 1) * 4].rearrange(
                                    "n p c -> p n c"), stage[:])
                            if qg == NQG - 1:
                                nc.gpsimd.collective_compute(
                                    "AllGather", OP.bypass,
                                    replica_groups=[list(range(NC))],
                                    ins=[agin[0, h].opt()],
                                    outs=[agout[0, h].opt()])
                        else:
                            nc.sync.dma_start(
                                agin[1, qg, h // 2][h % 2].rearrange(
                                    "n p c -> p n c"), stage[:])
                            if h % 2 == 1:
                                nc.gpsimd.collective_compute(
                                    "AllGather", OP.bypass,
                                    replica_groups=[list(range(NC))],
                                    ins=[agin[1, qg, h // 2].opt()],
                                    outs=[agout[1, qg, h // 2].opt()])
                        yield

            def _agt_fetch(b, tq):
                agt = sp.tile([128, NDX * 128], BF16, tag="ag",
                              bufs=AG_BUFS, name=f"ag{b}_{tq}")
                agt_v = agt[:].rearrange("p (n h c) -> p n h c",
                                         n=NC, h=HLOC, c=128)
                for hl in range(HLOC):
                    if b == 0:
                        src = agout[0, hl][:, tq]
                    else:
                        src = agout[1, tq // 4, hl // 2][:, hl % 2, tq % 4]
                    eng = nc.sync if hl % 2 == 0 else nc.scalar
                    eng.dma_start(agt_v[:, :, hl, :],
                                  src.rearrange("n p c -> p n c"))
                return agt

            def gen_oproj(b, tqs):
                for tq in tqs:
                    agt = _agt_fetch(b, tq)
                    pso = pp.tile([128, 512], F32, tag="mm512", bufs=2)
                    order = [core * HLOC + hl for hl in range(HLOC)
                             for core in range(NC)]
                    for j, i in enumerate(order):
                        _lb(nc.tensor.matmul(
                            pso[:], agt[:, i * 128:(i + 1) * 128],
                            wo_sl(i), start=(j == 0), stop=(j == NDX - 1)),
                            f"op{b}.t{tq}.{i}")
                    ost = sp.tile([128, 512], F32, tag="ost", bufs=1)
                    nc.vector.tensor_copy(ost[:], pso[:])
                    r0 = b * S + tq * 128
                    nc.sync.dma_start(out_d[r0:r0 + 128, :], ost[:])
                    yield

            # ---- software-pipelined emission ----
            g_qkv0 = gen_qkv(0)
            next(g_qkv0)
            emit_wvwo_loads()
            for _ in g_qkv0:
                pass
            g_att0, g_qkv1 = gen_att(0), gen_qkv(1)
            for _ in g_att0:
                for _ in range(3):
                    next(g_qkv1, None)
            for _ in g_qkv1:
                pass
            # att1: interleave most of o0; hold back the rest of o0 plus all
            # of o1 for the tail, which is then pure PE work that covers the
            # last collectives' wire time.
            g_att1 = gen_att(1)
            g_o0 = gen_oproj(0, list(range(NQT)))
            g_o1 = gen_oproj(1, list(range(NQT)))
            cnt = 0
            for _ in g_att1:
                cnt += 1
                if cnt >= 3:
                    next(g_o0, None)
            for _ in g_o0:
                pass
            for _ in g_o1:
                pass

    nc.compile()
    return nc


def kernel(**inputs):
    x = np.asarray(inputs["x"], np.float32)
    wq = np.asarray(inputs["wq"], np.float32)
    wk = np.asarray(inputs["wk"], np.float32)
    wv = np.asarray(inputs["wv"], np.float32)
    wo = np.asarray(inputs["wo"], np.float32)
    prompt = np.asarray(inputs["prompt"], np.float32)
    prompt_gate = np.asarray(inputs["prompt_gate"], np.float32)
    freqs_cos = np.asarray(inputs["freqs_cos"], np.float32)
    freqs_sin = np.asarray(inputs["freqs_sin"], np.float32)
    mask = np.asarray(inputs["mask"], np.float32)

    plan, mlist = _analyze_mask(mask)
    plan2 = _group_plan(plan, len(mlist))
    n_mt = len(mlist) + 1  # + trailing -inf tile
    plan_key = (tuple(plan2), n_mt)
    if plan_key not in _PROG_CACHE:
        _PROG_CACHE[plan_key] = _build_program(plan2, n_mt)
    nc = _PROG_CACHE[plan_key]

    # ---- shared host prep ----
    perm = np.concatenate([np.arange(0, HD, 2), np.arange(1, HD, 2)])
    xT = np.ascontiguousarray(x.reshape(T, D).T.astype(NPBF16))
    # [4, 128, NDX, 512]: [tcg, dx_in_block, dx_block, t_in_chunk]
    xt_tiles = np.ascontiguousarray(
        xT.reshape(NDX, 128, 4, 512).transpose(2, 1, 0, 3))
    ptT = np.ascontiguousarray(prompt.T.astype(NPBF16))       # [D, PL]
    pt_tiles = np.ascontiguousarray(
        ptT.reshape(NDX, 128, PL).transpose(1, 0, 2))
    cosT = np.ascontiguousarray(freqs_cos.T.astype(np.float32))
    sinT = np.ascontiguousarray(freqs_sin.T.astype(np.float32))
    neg = np.full((1, 128, 128), -1e30, np.float32)
    if mlist:
        mtiles = np.concatenate([np.stack(mlist), neg]).astype(NPBF16)
    else:
        mtiles = neg.astype(NPBF16)

    def shard_qk(w, c):
        rows = np.concatenate(
            [c * DLOC + j * HD + perm for j in range(HLOC)])
        wT = w[rows, :].T.astype(NPBF16)                      # [D, DLOC]
        return np.ascontiguousarray(
            wT.reshape(NDX, 128, HLOC, 128).transpose(2, 1, 0, 3))

    def shard_rhs(w, c):
        # rows c*DLOC..+DLOC of w, transposed -> [D, DLOC] -> [128,NDX,DLOC]
        wT = w[c * DLOC:(c + 1) * DLOC, :].T.astype(NPBF16)
        return np.ascontiguousarray(wT.reshape(NDX, 128, DLOC).transpose(1, 0, 2))

    in_maps = []
    for c in range(NC):
        in_maps.append(dict(
            xt=xt_tiles,
            wqt=shard_qk(wq, c),
            wkt=shard_qk(wk, c),
            wvt=shard_rhs(wv, c),
            wot=shard_rhs(wo, c),
            pt=pt_tiles,
            cosT=cosT,
            sinT=sinT,
            gates=np.ascontiguousarray(np.repeat(
                prompt_gate.reshape(H)[c * HLOC:(c + 1) * HLOC][None, :],
                PL, axis=0)).astype(np.float32),
            mtiles=mtiles,
            ident=np.eye(128, dtype=NPBF16),
        ))

    res = bass_utils.run_bass_kernel_spmd(
        nc, in_maps, core_ids=list(range(NC)),
        trace=bool(os.environ.get("BASS_TRACE")))
    kernel.last_result = res

    full = np.empty((T, D), np.float32)
    for c in range(NC):
        full[:, c * DLOC:(c + 1) * DLOC] = res.results[c]["out"]
    return full.reshape(B, S, D)
